# revision 62
# baseline (speedup 1.0000x reference)
"""Trainium2 Bass kernel for nn_LocalTransformer (4-layer transformer,
d=1024, 16 heads, dff=4096, seq=1024, batch=4, causal + 64-lookahead mask).

Sharding: 8 cores = 4 samples x 2 sequence halves; each core owns 512
tokens. Attention context is a relative window of 1152 positions
(p = t - qoff + 512); out-of-window positions are killed by per-core pad
biases added inside exp and affine_selects on the boundary chunks.

K/V exchange between half-pairs uses a ReduceScatter (each core stages its
own K/V into BOTH rank slots; the reduced output is own+peer and the
receiver subtracts its own contribution). The RS output is half the size
of the old AllGather output, halving collective time. Attention is split
into a local phase (own-token chunks 4-7, runs while the collectives fly)
and a remote phase (chunks 0-3, 8) that consumes the fixed-up K/V.

Precision: projections/FFN W1 in fp32r (full PE rate at N=512), V/W2
weights + attention inner products in bf16, activations/residual fp32.
"""
import numpy as np

L, D, H, DFF, S, B = 4, 1024, 16, 4096, 1024, 4
HD = D // H  # 64
T = 512  # local tokens per core
WIN = 1152  # kv window positions (9 chunks of 128)
NC = 9
EPS = 1e-5
NEG = -30000.0
V_E = H * 65  # 1040: per head [V(64) | denominator-ones col]

# packed per-layer params: columns of a [128, 96] tile
PC_BQ, PC_BK, PC_BO, PC_B2 = 0, 8, 16, 24
PC_L1G, PC_L1B, PC_L2G, PC_L2B = 32, 40, 48, 56
PC_B1 = 64  # 32 cols

_CACHE = {}


def _build_program():
    import concourse.bass as bass
    import concourse.tile as tile
    from concourse import bacc, mybir
    from contextlib import ExitStack

    f32, bf16, f32r = mybir.dt.float32, mybir.dt.bfloat16, mybir.dt.float32r
    AF = mybir.ActivationFunctionType
    ALU = mybir.AluOpType

    nc = bacc.Bacc("TRN2", target_bir_lowering=False, debug=False, num_devices=8)

    I = {}
    I["x0"] = nc.dram_tensor("x0", [D, T], f32r, kind="ExternalInput").ap()
    I["pb"] = nc.dram_tensor("pb", [NC, 128, 1], f32, kind="ExternalInput").ap()
    I["par"] = nc.dram_tensor("par", [L, 128, 96], f32, kind="ExternalInput").ap()
    # pre-laid weight panels (see _host_prep for layouts)
    I["wq_pan"] = nc.dram_tensor("wq_pan", [L, 8, 128, 8, 128], bf16, kind="ExternalInput").ap()
    I["wk_pan"] = nc.dram_tensor("wk_pan", [L, 8, 128, 8, 128], bf16, kind="ExternalInput").ap()
    I["wo_pan"] = nc.dram_tensor("wo_pan", [L, 8, 128, 8, 128], bf16, kind="ExternalInput").ap()
    I["w1_pan"] = nc.dram_tensor("w1_pan", [L, 32, 128, 8, 128], bf16, kind="ExternalInput").ap()
    I["w2_pan"] = nc.dram_tensor("w2_pan", [L, 8, 4, 128, 8, 128], bf16, kind="ExternalInput").ap()
    I["wv_pan"] = nc.dram_tensor("wv_pan", [L, 8, 128, V_E], bf16, kind="ExternalInput").ap()
    I["vbias"] = nc.dram_tensor("vbias", [L, 1, V_E], bf16, kind="ExternalInput").ap()
    I["ones1"] = nc.dram_tensor("ones1", [1, 128], f32r, kind="ExternalInput").ap()
    I["onesd"] = nc.dram_tensor("onesd", [128, 1], f32r, kind="ExternalInput").ap()
    I["onesrb"] = nc.dram_tensor("onesrb", [1, 512], bf16, kind="ExternalInput").ap()
    y = nc.dram_tensor("y", [D, T], f32, kind="ExternalOutput").ap()

    # ReduceScatter buffers: each core stages its own K/V into BOTH rank
    # halves; the reduced output is own+peer and the receiver subtracts its
    # own contribution. Split K/V collectives pipeline better than one
    # merged collective (K lands 45us earlier and unblocks remote QKs).
    rsk_in, rsk_out, rsv_in, rsv_out = [], [], [], []
    for l in range(L):
        rsk_in.append(nc.dram_tensor(f"rski{l}", [2 * D, 576], bf16, kind="Internal").ap())
        rsk_out.append(nc.dram_tensor(f"rsko{l}", [D, 576], bf16, kind="Internal").ap())
        rsv_in.append(nc.dram_tensor(f"rsvi{l}", [1152, V_E], bf16, kind="Internal").ap())
        rsv_out.append(nc.dram_tensor(f"rsvo{l}", [576, V_E], bf16, kind="Internal").ap())

    RG = [[0, 1], [2, 3], [4, 5], [6, 7]]
    SELEXT = {4: 64, 5: 192, 6: 320, 7: 448, 8: 512}
    LOCAL_CHUNKS = [4, 5, 6, 7]
    REMOTE_CHUNKS = [0, 1, 2, 3, 8]

    with tile.TileContext(nc) as tc, ExitStack() as ctx:
        pers = ctx.enter_context(tc.tile_pool(name="pers", bufs=1))
        X = [pers.tile([128, T], f32r, tag=f"X{i}", name=f"X{i}") for i in range(8)]
        XB = [pers.tile([128, T], bf16, tag=f"XB{i}", name=f"XB{i}") for i in range(8)]
        X2 = [pers.tile([128, T], f32r, tag=f"X2{i}", name=f"X2{i}") for i in range(8)]
        OP = [pers.tile([128, T], bf16, tag=f"OP{i}", name=f"OP{i}") for i in range(8)]
        Q = [pers.tile([128, T], bf16, tag=f"Q{i}", name=f"Qt{i}") for i in range(8)]
        KH = [pers.tile([128, WIN], bf16, tag=f"KH{i}", name=f"KHt{i}") for i in range(8)]
        VT = [pers.tile([128, V_E], bf16, tag=f"VT{i}", name=f"VTt{i}") for i in range(NC)]
        AVL = [pers.tile([128, T], bf16, tag=f"AVL{i}", name=f"AVLt{i}") for i in range(16)]
        HT = [pers.tile([128, T], bf16, tag=f"HT{i}", name=f"HTt{i}") for i in range(32)]
        ones1 = pers.tile([1, 128], f32r, tag="ones1", name="ones1t")
        onesd_t = pers.tile([128, 1], f32r, tag="onesd", name="onesdt")
        onesrb_t = pers.tile([1, T], bf16, tag="onesrb", name="onesrbt")
        pb_t = [pers.tile([128, 1], f32, tag=f"pb{i}", name=f"pbt{i}") for i in range(NC)]

        wp = ctx.enter_context(tc.tile_pool(name="wp", bufs=3))  # [128,8,128] panels
        vwp = ctx.enter_context(tc.tile_pool(name="vwp", bufs=8))  # V weight panels
        smw = ctx.enter_context(tc.tile_pool(name="smw", bufs=1))  # small weights
        pp = ctx.enter_context(tc.tile_pool(name="pp", bufs=2, space="PSUM"))
        pp2 = ctx.enter_context(tc.tile_pool(name="pp2", bufs=2, space="PSUM"))
        pav = ctx.enter_context(tc.tile_pool(name="pav", bufs=2, space="PSUM"))
        prA = ctx.enter_context(tc.tile_pool(name="prA", bufs=4))  # probs pairs
        prB = ctx.enter_context(tc.tile_pool(name="prB", bufs=2))  # probs singles
        tps = ctx.enter_context(tc.tile_pool(name="tps", bufs=4))  # [128,T] f32/bf16
        tpr = ctx.enter_context(tc.tile_pool(name="tpr", bufs=2))  # [128,T] f32r
        fx = ctx.enter_context(tc.tile_pool(name="fx", bufs=2))  # fixup bf16 loads
        sm = ctx.enter_context(tc.tile_pool(name="sm", bufs=2))  # [1,T] smalls
        rhp = ctx.enter_context(tc.tile_pool(name="rhp", bufs=1))  # [1,T] recip
        pcp = ctx.enter_context(tc.tile_pool(name="pcp", bufs=4))  # params [128,96]

        r = lambda ap: ap.bitcast(f32r)
        q32 = lambda ap: ap.bitcast(f32)

        nc.sync.dma_start(out=ones1[:], in_=I["ones1"][:])
        nc.sync.dma_start(out=onesd_t[:], in_=I["onesd"][:])
        nc.sync.dma_start(out=onesrb_t[:], in_=I["onesrb"][:])
        for i in range(NC):
            nc.sync.dma_start(out=pb_t[i][:], in_=I["pb"][i])
        for i in range(8):
            nc.sync.dma_start(out=X[i][:], in_=I["x0"][i * 128 : (i + 1) * 128, :])
        for i in range(8):
            nc.gpsimd.memset(KH[i][:], 0.0)
        for i in range(NC):
            nc.gpsimd.memset(VT[i][:], 0.0)
        for i in range(8):
            nc.scalar.activation(XB[i][:], q32(X[i][:]), AF.Copy)

        def wpanel(src5d, l, ec):
            pan = wp.tile([128, 8, 128], bf16, tag="wpan", name="wpan")
            nc.sync.dma_start(out=pan[:], in_=src5d[l, ec])
            return pan

        for l in range(L):
            par = pcp.tile([128, 96], f32, tag="par", name="par")
            nc.sync.dma_start(out=par[:], in_=I["par"][l])

            # ---------- K projection -> KH[:, 512:1024], stage both halves ----------
            for ec in range(8):
                pan = wpanel(I["wk_pan"], l, ec)
                ps = pp.tile([128, T], f32, tag="ps", name="ps")
                for dc in range(8):
                    nc.tensor.matmul(
                        out=ps[:], lhsT=pan[:, dc, :], rhs=XB[dc][:],
                        start=(dc == 0), stop=(dc == 7),
                    )
                nc.scalar.activation(
                    KH[ec][:, 512:1024], ps[:], AF.Identity,
                    bias=par[:, PC_BK + ec : PC_BK + ec + 1], scale=1.0,
                )
                with tc.high_priority():
                    for half in (0, D):
                        nc.sync.dma_start(
                            out=rsk_in[l][half + ec * 128 : half + (ec + 1) * 128, 0:512],
                            in_=KH[ec][:, 512:1024],
                        )
                        nc.sync.dma_start(
                            out=rsk_in[l][half + ec * 128 : half + (ec + 1) * 128, 512:576],
                            in_=KH[ec][:, 512:576],
                        )
            nc.gpsimd.collective_compute(
                "ReduceScatter", mybir.AluOpType.add, replica_groups=RG,
                ins=[rsk_in[l][:]], outs=[rsk_out[l][:]],
            )

            # ---------- V projection (token-major), stage both halves ----------
            vb = smw.tile([1, V_E], bf16, tag="vbias", name="vbias")
            nc.sync.dma_start(out=vb[:], in_=I["vbias"][l])
            for s0, s1 in ((0, 512), (512, 1024), (1024, V_E)):
                w = s1 - s0
                vpans = []
                for dc in range(8):
                    vp = vwp.tile([128, 512], bf16, tag="vpan", name="vpan")
                    nc.sync.dma_start(out=vp[:, 0:w], in_=I["wv_pan"][l, dc, :, s0:s1])
                    vpans.append(vp)
                for tcx in range(4):
                    ps = pp.tile([128, T], f32, tag="ps", name="ps")
                    for dc in range(8):
                        nc.tensor.matmul(
                            out=ps[:, 0:w],
                            lhsT=XB[dc][:, tcx * 128 : (tcx + 1) * 128],
                            rhs=vpans[dc][:, 0:w],
                            start=(dc == 0), stop=False,
                        )
                    nc.tensor.matmul(
                        out=ps[:, 0:w],
                        lhsT=onesrb_t[:, tcx * 128 : (tcx + 1) * 128],
                        rhs=vb[:, s0:s1],
                        start=False, stop=True,
                    )
                    nc.scalar.activation(VT[4 + tcx][:, s0:s1], ps[:, 0:w], AF.Copy)
            with tc.high_priority():
                for tcx in range(4):
                    for half in (0, 576):
                        nc.sync.dma_start(
                            out=rsv_in[l][half + tcx * 128 : half + (tcx + 1) * 128, :],
                            in_=VT[4 + tcx][:],
                        )
                for half in (0, 576):
                    nc.sync.dma_start(
                        out=rsv_in[l][half + 512 : half + 576, :], in_=VT[4][0:64, :]
                    )
            nc.gpsimd.collective_compute(
                "ReduceScatter", mybir.AluOpType.add, replica_groups=RG,
                ins=[rsv_in[l][:]], outs=[rsv_out[l][:]],
            )
            for pc in range(4, 8):  # local denominator ones-columns
                nc.gpsimd.memset(
                    VT[pc][:].rearrange("p (h c) -> p h c", c=65)[:, :, 64:65], 1.0
                )

            # ---------- Q projection ----------
            for ec in range(8):
                pan = wpanel(I["wq_pan"], l, ec)
                ps = pp.tile([128, T], f32, tag="ps", name="ps")
                for dc in range(8):
                    nc.tensor.matmul(
                        out=ps[:], lhsT=pan[:, dc, :], rhs=XB[dc][:],
                        start=(dc == 0), stop=(dc == 7),
                    )
                nc.scalar.activation(
                    Q[ec][:], ps[:], AF.Identity,
                    bias=par[:, PC_BQ + ec : PC_BQ + ec + 1], scale=1.0,
                )

            # ---------- attention: software-pipelined QK/exp ahead of AV ----------
            # units are chunk pairs sharing one 2-bank PSUM tile and ONE exp
            # (pb bias is identical within each pair for both core parities)
            LOOKU = 3  # units of QK/exp lookahead ahead of the AV consumer

            def attn_pipeline(units, close_head):
                from collections import deque

                work = [(h, ui) for h in range(16) for ui in range(len(units))]
                nu = len(units)
                nch = sum(len(u) for u in units)
                pend = deque()
                avs = {}

                def drain_one():
                    h2, ui2, pts = pend.popleft()
                    if ui2 == 0:
                        avs[h2] = pav.tile([128, T], f32, tag="av", name="av")
                    base = sum(len(units[j]) for j in range(ui2))
                    for k, (c2, ptk) in enumerate(pts):
                        nc.tensor.matmul(
                            out=avs[h2][0:65, :],
                            lhsT=VT[c2][:, h2 * 65 : h2 * 65 + 65],
                            rhs=ptk,
                            start=(base + k == 0), stop=(base + k == nch - 1),
                        )
                    if ui2 == nu - 1:
                        close_head(h2, avs.pop(h2))

                for h, ui in work:
                    chunks_u = units[ui]
                    par_, kc = h % 2, h // 2
                    rows = slice(par_ * 64, par_ * 64 + 64)
                    w = 512 * len(chunks_u)
                    if len(chunks_u) == 2:
                        sc = pp2.tile([128, 2 * T], f32, tag="ps2", name="ps2")
                        pt = prA.tile([128, 2 * T], bf16, tag="probs2", name="probs2")
                    else:
                        sc = pp.tile([128, T], f32, tag="ps", name="ps")
                        pt = prB.tile([128, T], bf16, tag="probs", name="probs")
                    for k, c in enumerate(chunks_u):
                        nc.tensor.matmul(
                            out=sc[:, k * 512 : (k + 1) * 512],
                            lhsT=KH[kc][rows, c * 128 : (c + 1) * 128],
                            rhs=Q[kc][rows, :],
                            start=True, stop=True,
                        )
                    nc.scalar.activation(
                        pt[:, 0:w], sc[:, 0:w], AF.Exp,
                        bias=pb_t[chunks_u[0]][:], scale=1.0,
                    )
                    pts = []
                    for k, c in enumerate(chunks_u):
                        ptk = pt[:, k * 512 : (k + 1) * 512]
                        if c in SELEXT:
                            ext = SELEXT[c]
                            nc.gpsimd.affine_select(
                                out=ptk[:, 0:ext], in_=ptk[:, 0:ext],
                                pattern=[[1, ext]], compare_op=ALU.is_ge,
                                fill=0.0, base=576 - c * 128, channel_multiplier=-1,
                            )
                        pts.append((c, ptk))
                    pend.append((h, ui, pts))
                    if len(pend) > LOOKU:
                        drain_one()
                while pend:
                    drain_one()

            attn_pipeline(
                [(4, 5), (6, 7)],
                lambda h, av: nc.vector.tensor_copy(AVL[h][0:65, :], av[0:65, :]),
            )

            # ---------- K/V fixup (consumes RS results) ----------
            for ec in range(8):
                ka = fx.tile([128, 576], bf16, tag="fxk", name="fxk")
                nc.sync.dma_start(out=ka[:], in_=rsk_out[l][ec * 128 : (ec + 1) * 128, :])
                nc.vector.tensor_sub(
                    KH[ec][:, 0:512], ka[:, 0:512], KH[ec][:, 512:1024]
                )
                nc.vector.tensor_sub(
                    KH[ec][:, 1024:1088], ka[:, 512:576], KH[ec][:, 512:576]
                )
            for pc in range(4):
                va = fx.tile([128, V_E], bf16, tag="fxv", name="fxv")
                nc.sync.dma_start(out=va[:], in_=rsv_out[l][pc * 128 : (pc + 1) * 128, :])
                nc.vector.tensor_sub(VT[pc][:], va[:], VT[4 + pc][:])
            va = fx.tile([128, V_E], bf16, tag="fxv", name="fxv")
            nc.sync.dma_start(out=va[0:64, :], in_=rsv_out[l][512:576, :])
            nc.vector.tensor_sub(VT[8][0:64, :], va[0:64, :], VT[4][0:64, :])
            for pc in (0, 1, 2, 3, 8):
                nc.gpsimd.memset(
                    VT[pc][:].rearrange("p (h c) -> p h c", c=65)[:, :, 64:65], 1.0
                )

            # ---------- attention phase B: remote chunks + deferred combine ----------
            def close_b(h, av2):
                avt = tps.tile([128, T], f32, tag="t512", name="t512")
                nc.vector.tensor_add(avt[0:65, :], av2[0:65, :], AVL[h][0:65, :])
                rr = rhp.tile([1, T], f32r, tag="rh", name="rht")
                with nc.allow_low_precision(reason="f32r is fp32-width storage"):
                    nc.vector.reciprocal(rr[:], avt[64:65, :])
                bc = pp.tile([128, T], f32, tag="ps", name="ps")
                nc.tensor.matmul(
                    out=bc[0:64, :], lhsT=r(ones1[:, 0:64]), rhs=r(rr[:]),
                    start=True, stop=True,
                )
                if h % 2 == 0:
                    nc.vector.tensor_mul(
                        OP[h // 2][0:64, :], avt[0:64, :], bc[0:64, :]
                    )
                else:
                    # stash in the (now free) AVL tile; DMA'd to OP rows 64:128
                    # after phase B so SP prefetching is not blocked mid-phase
                    nc.vector.tensor_mul(AVL[h][0:64, :], avt[0:64, :], bc[0:64, :])

            attn_pipeline([(0, 1), (2, 3), (8,)], close_b)
            for h in range(1, 16, 2):
                nc.sync.dma_start(out=OP[h // 2][64:128, :], in_=AVL[h][0:64, :])

            def layernorm(src, gcol, bcol, dst, make_xb=False):
                mu = pp.tile([1, T], f32, tag="ps", name="ps")
                ms = pp.tile([1, T], f32, tag="ps", name="ps")
                for dc in range(8):
                    sq = tpr.tile([128, T], f32r, tag="sqr", name="sqr")
                    nc.scalar.square(sq[:], q32(src[dc][:]))
                    nc.tensor.matmul(
                        out=mu[:], lhsT=r(onesd_t[:]), rhs=r(src[dc][:]),
                        start=(dc == 0), stop=(dc == 7),
                    )
                    nc.tensor.matmul(
                        out=ms[:], lhsT=r(onesd_t[:]), rhs=r(sq[:]),
                        start=(dc == 0), stop=(dc == 7),
                    )
                mu_sb = sm.tile([1, T], f32r, tag="sm1", name="mu")
                nc.vector.tensor_copy(mu_sb[:], mu[:])
                t2 = sm.tile([1, T], f32, tag="sm1", name="t2")
                nc.vector.tensor_mul(t2[:], q32(mu_sb[:]), q32(mu_sb[:]))
                var = sm.tile([1, T], f32, tag="sm1", name="var")
                nc.vector.tensor_sub(var[:], ms[:], t2[:])
                nc.vector.tensor_scalar_add(var[:], var[:], EPS)
                std = sm.tile([1, T], f32, tag="sm1", name="std")
                nc.scalar.sqrt(std[:], var[:])
                rstd = sm.tile([1, T], f32r, tag="sm1", name="rstd")
                with nc.allow_low_precision(reason="f32r is fp32-width storage"):
                    nc.vector.reciprocal(rstd[:], std[:])
                mub = pp.tile([128, T], f32, tag="ps", name="ps")
                nc.tensor.matmul(
                    out=mub[:], lhsT=r(ones1[:]), rhs=r(mu_sb[:]), start=True, stop=True
                )
                rsb = pp.tile([128, T], f32, tag="ps", name="ps")
                nc.tensor.matmul(
                    out=rsb[:], lhsT=r(ones1[:]), rhs=r(rstd[:]), start=True, stop=True
                )
                rsb_sb = tps.tile([128, T], f32, tag="t512", name="t512")
                nc.vector.tensor_copy(rsb_sb[:], rsb[:])
                for dc in range(8):
                    t = tps.tile([128, T], f32, tag="t512", name="t512")
                    nc.vector.tensor_sub(t[:], q32(src[dc][:]), mub[:])
                    t2b = tps.tile([128, T], f32, tag="t512", name="t512")
                    nc.vector.tensor_mul(t2b[:], t[:], rsb_sb[:])
                    nc.scalar.activation(
                        dst[dc][:], t2b[:], AF.Identity,
                        bias=par[:, bcol + dc : bcol + dc + 1],
                        scale=par[:, gcol + dc : gcol + dc + 1],
                    )
                    if make_xb:
                        nc.scalar.activation(XB[dc][:], q32(dst[dc][:]), AF.Copy)

            # ---------- Wo + residual + LN1 ----------
            for ec in range(8):
                pan = wp.tile([128, 8, 128], bf16, tag="wpan2", name="wpan2")
                nc.sync.dma_start(out=pan[:], in_=I["wo_pan"][l, ec])
                ps = pp.tile([128, T], f32, tag="ps", name="ps")
                for dc in range(8):
                    nc.tensor.matmul(
                        out=ps[:], lhsT=pan[:, dc, :], rhs=OP[dc][:],
                        start=(dc == 0), stop=(dc == 7),
                    )
                nc.vector.scalar_tensor_tensor(
                    out=X2[ec][:], in0=ps[:], scalar=par[:, PC_BO + ec : PC_BO + ec + 1],
                    in1=q32(X[ec][:]), op0=ALU.add, op1=ALU.add,
                )
            layernorm(X2, PC_L1G, PC_L1B, X, make_xb=True)

            # ---------- FFN: W1 -> H (bf16), W2 accumulated in PSUM ----------
            for fc in range(32):
                pan = wp.tile([128, 8, 128], bf16, tag="wpan", name="wpan")
                nc.sync.dma_start(out=pan[:], in_=I["w1_pan"][l, fc])
                ps = pp.tile([128, T], f32, tag="ps", name="ps")
                for dc in range(8):
                    nc.tensor.matmul(
                        out=ps[:], lhsT=pan[:, dc, :], rhs=XB[dc][:],
                        start=(dc == 0), stop=(dc == 7),
                    )
                nc.scalar.activation(
                    HT[fc][:], ps[:], AF.Relu,
                    bias=par[:, PC_B1 + fc : PC_B1 + fc + 1], scale=1.0,
                )
            for ec in range(8):
                ps = pp.tile([128, T], f32, tag="ps", name="ps")
                for g in range(4):
                    pan = wp.tile([128, 8, 128], bf16, tag="wpan2", name="wpan2")
                    nc.sync.dma_start(out=pan[:], in_=I["w2_pan"][l, ec, g])
                    for k in range(8):
                        fc = g * 8 + k
                        nc.tensor.matmul(
                            out=ps[:], lhsT=pan[:, k, :], rhs=HT[fc][:],
                            start=(fc == 0), stop=(fc == 31),
                        )
                nc.vector.scalar_tensor_tensor(
                    out=X2[ec][:], in0=ps[:], scalar=par[:, PC_B2 + ec : PC_B2 + ec + 1],
                    in1=q32(X[ec][:]), op0=ALU.add, op1=ALU.add,
                )
            layernorm(X2, PC_L2G, PC_L2B, X, make_xb=(l < L - 1))

        for ec in range(8):
            nc.sync.dma_start(out=y[ec * 128 : (ec + 1) * 128, :], in_=q32(X[ec][:]))

    nc.compile()
    return nc


def _to_bf16(a):
    try:
        import ml_dtypes

        return a.astype(ml_dtypes.bfloat16)
    except ImportError:
        u = a.astype(np.float32).view(np.uint32)
        return ((u + 0x7FFF + ((u >> 16) & 1)) >> 16).astype(np.uint16)


def _host_prep(inputs):
    g = {}
    Wqkv = np.asarray(inputs["Wqkv"], np.float32)
    bqkv = np.asarray(inputs["bqkv"], np.float32)
    sc = 1.0 / np.sqrt(HD)
    wvT = np.zeros((L, D, V_E), np.float32)
    vbias = np.zeros((L, 1, V_E), np.float32)
    wq = np.zeros((L, D, D), np.float32)
    wk = np.zeros((L, D, D), np.float32)
    for l in range(L):
        Wq, Wk, Wv = Wqkv[l, 0:D], Wqkv[l, D : 2 * D], Wqkv[l, 2 * D :]
        bv = bqkv[l, 2 * D :]
        wq[l] = Wq.T * sc
        wk[l] = Wk.T
        for h in range(H):
            off = h * 65
            wvT[l, :, off : off + 64] = Wv.T[:, h * 64 : h * 64 + 64]
            vbias[l, 0, off : off + 64] = bv[h * 64 : h * 64 + 64]

    def pan5(wT):  # [L, D, M] -> [L, M/128, 128, 8, 128]
        Lx, Dx, M = wT.shape
        return np.ascontiguousarray(
            wT.reshape(Lx, 8, 128, M // 128, 128).transpose(0, 3, 2, 1, 4)
        )

    woT = np.asarray(inputs["Wo"], np.float32).transpose(0, 2, 1)
    w1T = np.asarray(inputs["W1"], np.float32).transpose(0, 2, 1)
    w2T = np.asarray(inputs["W2"], np.float32).transpose(0, 2, 1)
    g["wq_pan"] = _to_bf16(pan5(wq))
    g["wk_pan"] = _to_bf16(pan5(wk))
    g["wo_pan"] = _to_bf16(pan5(woT))
    g["w1_pan"] = _to_bf16(pan5(w1T))
    # w2: [L, DFF, D] -> [L, ec, g, 128, k, 128]
    w2p = np.empty((L, 8, 4, 128, 8, 128), np.float32)
    for ec in range(8):
        for gg in range(4):
            for k in range(8):
                fc = gg * 8 + k
                w2p[:, ec, gg, :, k, :] = w2T[
                    :, fc * 128 : (fc + 1) * 128, ec * 128 : (ec + 1) * 128
                ]
    g["w2_pan"] = _to_bf16(np.ascontiguousarray(w2p))
    g["wv_pan"] = _to_bf16(
        np.ascontiguousarray(wvT.reshape(L, 8, 128, V_E))
    )
    g["vbias"] = _to_bf16(vbias)

    par = np.zeros((L, 128, 96), np.float32)
    par[:, :, PC_BQ : PC_BQ + 8] = (bqkv[:, 0:D] * sc).reshape(L, 8, 128).transpose(0, 2, 1)
    par[:, :, PC_BK : PC_BK + 8] = bqkv[:, D : 2 * D].reshape(L, 8, 128).transpose(0, 2, 1)
    par[:, :, PC_BO : PC_BO + 8] = np.asarray(inputs["bo"], np.float32).reshape(L, 8, 128).transpose(0, 2, 1)
    par[:, :, PC_B2 : PC_B2 + 8] = np.asarray(inputs["b2"], np.float32).reshape(L, 8, 128).transpose(0, 2, 1)
    par[:, :, PC_L1G : PC_L1G + 8] = np.asarray(inputs["g1"], np.float32).reshape(L, 8, 128).transpose(0, 2, 1)
    par[:, :, PC_L1B : PC_L1B + 8] = np.asarray(inputs["be1"], np.float32).reshape(L, 8, 128).transpose(0, 2, 1)
    par[:, :, PC_L2G : PC_L2G + 8] = np.asarray(inputs["g2"], np.float32).reshape(L, 8, 128).transpose(0, 2, 1)
    par[:, :, PC_L2B : PC_L2B + 8] = np.asarray(inputs["be2"], np.float32).reshape(L, 8, 128).transpose(0, 2, 1)
    par[:, :, PC_B1 : PC_B1 + 32] = np.asarray(inputs["b1"], np.float32).reshape(L, 32, 128).transpose(0, 2, 1)
    g["par"] = par

    g["ones1"] = np.ones((1, 128), np.float32)
    g["onesd"] = np.full((128, 1), 1.0 / D, np.float32)
    g["onesrb"] = _to_bf16(np.ones((1, 512), np.float32))

    xb = np.asarray(inputs["x"], np.float32).transpose(1, 0, 2)
    in_maps = []
    for c in range(8):
        b, hh = c // 2, c % 2
        pb = np.zeros((NC, 128, 1), np.float32)
        if hh == 0:
            pb[0:4] = NEG
            pb[8, 64:128] = NEG
        else:
            pb[8] = NEG
        m = dict(g)
        m["x0"] = np.ascontiguousarray(xb[b, hh * T : (hh + 1) * T, :].T)
        m["pb"] = pb
        in_maps.append(m)
    return in_maps


def kernel(**inputs):
    from concourse.bass_utils import run_bass_kernel_spmd

    if "nc" not in _CACHE:
        _CACHE["nc"] = _build_program()
    nc = _CACHE["nc"]
    in_maps = _host_prep(inputs)
    res = run_bass_kernel_spmd(nc, in_maps, core_ids=list(range(8)))
    out = np.zeros((S, B, D), np.float32)
    for c in range(8):
        b, hh = c // 2, c % 2
        out[hh * T : (hh + 1) * T, b, :] = res.results[c]["y"].T
    return out


# revision 81
# speedup vs baseline: 1.0459x; 1.0459x over previous
"""Trainium2 Bass kernel for nn_LocalTransformer (4-layer transformer,
d=1024, 16 heads, dff=4096, seq=1024, batch=4, causal + 64-lookahead mask).

Sharding: 8 cores = 4 samples x 2 sequence halves; each core owns 512
tokens. Attention context is a relative window of 1152 positions
(p = t - qoff + 512); out-of-window positions are killed by per-core pad
biases added inside exp and affine_selects on the boundary chunks.

K/V exchange between half-pairs uses ReduceScatter (each core stages its
own K/V into BOTH rank slots; the reduced output is own+peer and the
receiver subtracts its own contribution), half the output bytes of the
AllGather equivalent. Attention is split into a local phase (own-token
chunks 4-7, runs while the collectives fly) and a remote phase (chunks
0-3 and 8) that consumes the fixed-up K/V. Both phases run as a software
pipeline: QK/exp run several chunk-pairs ahead of the AV consumer, with
two score chunks packed into one 2-bank PSUM tile so a single Act-engine
exp instruction covers them (the Act engine is the attention bottleneck).

Weights are pre-laid host-side into SBUF panel layout (fully contiguous
DMA descriptors) in bf16; per-layer biases/LN params are packed into one
[128, 96] tensor loaded with a single DMA. The FFN materializes
H = relu(W1 x) as bf16 tiles and accumulates all 32 W2 partial matmuls
in PSUM. Activations/residual stay fp32; matmul rhs uses bf16 copies XB.
"""
import numpy as np

L, D, H, DFF, S, B = 4, 1024, 16, 4096, 1024, 4
HD = D // H  # 64
T = 512  # local tokens per core
WIN = 1152  # kv window positions (9 chunks of 128)
NC = 9
EPS = 1e-5
NEG = -30000.0
V_E = H * 65  # 1040: per head [V(64) | denominator-ones col]

# packed per-layer params: columns of a [128, 96] tile
PC_BQ, PC_BK, PC_BO, PC_B2 = 0, 8, 16, 24
PC_L1G, PC_L1B, PC_L2G, PC_L2B = 32, 40, 48, 56
PC_B1 = 64  # 32 cols

_CACHE = {}


def _build_program():
    import concourse.bass as bass
    import concourse.tile as tile
    from concourse import bacc, mybir
    from contextlib import ExitStack

    f32, bf16, f32r = mybir.dt.float32, mybir.dt.bfloat16, mybir.dt.float32r
    AF = mybir.ActivationFunctionType
    ALU = mybir.AluOpType

    nc = bacc.Bacc("TRN2", target_bir_lowering=False, debug=False, num_devices=8)

    I = {}
    I["x0"] = nc.dram_tensor("x0", [D, T], f32r, kind="ExternalInput").ap()
    I["pb"] = nc.dram_tensor("pb", [NC, 128, 1], f32, kind="ExternalInput").ap()
    I["par"] = nc.dram_tensor("par", [L, 128, 96], f32, kind="ExternalInput").ap()
    # pre-laid weight panels (see _host_prep for layouts)
    I["wq_pan"] = nc.dram_tensor("wq_pan", [L, 8, 128, 8, 128], bf16, kind="ExternalInput").ap()
    I["wk_pan"] = nc.dram_tensor("wk_pan", [L, 8, 128, 8, 128], bf16, kind="ExternalInput").ap()
    I["wo_pan"] = nc.dram_tensor("wo_pan", [L, 8, 128, 8, 128], bf16, kind="ExternalInput").ap()
    I["w1_pan"] = nc.dram_tensor("w1_pan", [L, 32, 128, 8, 128], bf16, kind="ExternalInput").ap()
    I["w2_pan"] = nc.dram_tensor("w2_pan", [L, 8, 4, 128, 8, 128], bf16, kind="ExternalInput").ap()
    I["wv_pan"] = nc.dram_tensor("wv_pan", [L, 8, 128, V_E], bf16, kind="ExternalInput").ap()
    I["vbias"] = nc.dram_tensor("vbias", [L, 1, V_E], bf16, kind="ExternalInput").ap()
    I["ones1"] = nc.dram_tensor("ones1", [1, 128], f32r, kind="ExternalInput").ap()
    I["onesd"] = nc.dram_tensor("onesd", [128, 1], f32r, kind="ExternalInput").ap()
    I["onesrb"] = nc.dram_tensor("onesrb", [1, 512], bf16, kind="ExternalInput").ap()
    y = nc.dram_tensor("y", [D, T], f32, kind="ExternalOutput").ap()

    # ReduceScatter buffers: each core stages its own K/V into BOTH rank
    # halves; the reduced output is own+peer and the receiver subtracts its
    # own contribution. Split K/V collectives pipeline better than one
    # merged collective (K lands 45us earlier and unblocks remote QKs).
    rsk_in, rsk_out, rsv_in, rsv_out = [], [], [], []
    for l in range(L):
        rsk_in.append(nc.dram_tensor(f"rski{l}", [2 * D, 576], bf16, kind="Internal").ap())
        rsk_out.append(nc.dram_tensor(f"rsko{l}", [D, 576], bf16, kind="Internal").ap())
        rsv_in.append(nc.dram_tensor(f"rsvi{l}", [1152, V_E], bf16, kind="Internal").ap())
        rsv_out.append(nc.dram_tensor(f"rsvo{l}", [576, V_E], bf16, kind="Internal").ap())

    RG = [[0, 1], [2, 3], [4, 5], [6, 7]]
    SELEXT = {4: 64, 5: 192, 6: 320, 7: 448, 8: 512}

    with tile.TileContext(nc) as tc, ExitStack() as ctx:
        pers = ctx.enter_context(tc.tile_pool(name="pers", bufs=1))
        X = [pers.tile([128, T], f32r, tag=f"X{i}", name=f"X{i}") for i in range(8)]
        XB = [pers.tile([128, T], bf16, tag=f"XB{i}", name=f"XB{i}") for i in range(8)]
        X2 = [pers.tile([128, T], f32r, tag=f"X2{i}", name=f"X2{i}") for i in range(8)]
        OP = [pers.tile([128, T], bf16, tag=f"OP{i}", name=f"OP{i}") for i in range(8)]
        Q = [pers.tile([128, T], bf16, tag=f"Q{i}", name=f"Qt{i}") for i in range(8)]
        KH = [pers.tile([128, WIN], bf16, tag=f"KH{i}", name=f"KHt{i}") for i in range(8)]
        VT = [pers.tile([128, V_E], bf16, tag=f"VT{i}", name=f"VTt{i}") for i in range(NC)]
        AVL = [pers.tile([128, T], bf16, tag=f"AVL{i}", name=f"AVLt{i}") for i in range(16)]
        HT = [pers.tile([128, T], bf16, tag=f"HT{i}", name=f"HTt{i}") for i in range(32)]
        ones1 = pers.tile([1, 128], f32r, tag="ones1", name="ones1t")
        onesd_t = pers.tile([128, 1], f32r, tag="onesd", name="onesdt")
        onesrb_t = pers.tile([1, T], bf16, tag="onesrb", name="onesrbt")
        pb_t = [pers.tile([128, 1], f32, tag=f"pb{i}", name=f"pbt{i}") for i in range(NC)]

        wp = ctx.enter_context(tc.tile_pool(name="wp", bufs=3))  # [128,8,128] panels
        vwp = ctx.enter_context(tc.tile_pool(name="vwp", bufs=8))  # V weight panels
        smw = ctx.enter_context(tc.tile_pool(name="smw", bufs=1))  # small weights
        pp = ctx.enter_context(tc.tile_pool(name="pp", bufs=2, space="PSUM"))
        pp2 = ctx.enter_context(tc.tile_pool(name="pp2", bufs=2, space="PSUM"))
        pav = ctx.enter_context(tc.tile_pool(name="pav", bufs=2, space="PSUM"))
        prA = ctx.enter_context(tc.tile_pool(name="prA", bufs=6))  # probs pairs
        prB = ctx.enter_context(tc.tile_pool(name="prB", bufs=2))  # probs singles
        tps = ctx.enter_context(tc.tile_pool(name="tps", bufs=4))  # [128,T] f32/bf16
        tpr = ctx.enter_context(tc.tile_pool(name="tpr", bufs=2))  # [128,T] f32r
        fx = ctx.enter_context(tc.tile_pool(name="fx", bufs=2))  # fixup bf16 loads
        sm = ctx.enter_context(tc.tile_pool(name="sm", bufs=2))  # [1,T] smalls
        rhp = ctx.enter_context(tc.tile_pool(name="rhp", bufs=1))  # [1,T] recip
        pcp = ctx.enter_context(tc.tile_pool(name="pcp", bufs=4))  # params [128,96]

        r = lambda ap: ap.bitcast(f32r)
        q32 = lambda ap: ap.bitcast(f32)

        nc.sync.dma_start(out=ones1[:], in_=I["ones1"][:])
        nc.sync.dma_start(out=onesd_t[:], in_=I["onesd"][:])
        nc.sync.dma_start(out=onesrb_t[:], in_=I["onesrb"][:])
        for i in range(NC):
            nc.sync.dma_start(out=pb_t[i][:], in_=I["pb"][i])
        for i in range(8):
            nc.sync.dma_start(out=X[i][:], in_=I["x0"][i * 128 : (i + 1) * 128, :])
        for i in range(8):
            nc.gpsimd.memset(KH[i][:], 0.0)
        for i in range(NC):
            nc.gpsimd.memset(VT[i][:], 0.0)
        for i in range(8):
            nc.scalar.activation(XB[i][:], q32(X[i][:]), AF.Copy)

        def wpanel(src5d, l, ec):
            pan = wp.tile([128, 8, 128], bf16, tag="wpan", name="wpan")
            nc.sync.dma_start(out=pan[:], in_=src5d[l, ec])
            return pan

        for l in range(L):
            par = pcp.tile([128, 96], f32, tag="par", name="par")
            nc.sync.dma_start(out=par[:], in_=I["par"][l])

            # ---------- K projection -> KH[:, 512:1024], stage both halves ----------
            for ec in range(8):
                pan = wpanel(I["wk_pan"], l, ec)
                ps = pp.tile([128, T], f32, tag="ps", name="ps")
                for dc in range(8):
                    nc.tensor.matmul(
                        out=ps[:], lhsT=pan[:, dc, :], rhs=XB[dc][:],
                        start=(dc == 0), stop=(dc == 7),
                    )
                nc.scalar.activation(
                    KH[ec][:, 512:1024], ps[:], AF.Identity,
                    bias=par[:, PC_BK + ec : PC_BK + ec + 1], scale=1.0,
                )
                with tc.high_priority():
                    for half in (0, D):
                        nc.sync.dma_start(
                            out=rsk_in[l][half + ec * 128 : half + (ec + 1) * 128, 0:512],
                            in_=KH[ec][:, 512:1024],
                        )
                        nc.sync.dma_start(
                            out=rsk_in[l][half + ec * 128 : half + (ec + 1) * 128, 512:576],
                            in_=KH[ec][:, 512:576],
                        )
            nc.gpsimd.collective_compute(
                "ReduceScatter", mybir.AluOpType.add, replica_groups=RG,
                ins=[rsk_in[l][:]], outs=[rsk_out[l][:]],
            )

            # ---------- V projection (token-major), stage both halves ----------
            vb = smw.tile([1, V_E], bf16, tag="vbias", name="vbias")
            nc.sync.dma_start(out=vb[:], in_=I["vbias"][l])
            for s0, s1 in ((0, 512), (512, 1024), (1024, V_E)):
                w = s1 - s0
                vpans = []
                for dc in range(8):
                    vp = vwp.tile([128, 512], bf16, tag="vpan", name="vpan")
                    nc.sync.dma_start(out=vp[:, 0:w], in_=I["wv_pan"][l, dc, :, s0:s1])
                    vpans.append(vp)
                for tcx in range(4):
                    ps = pp.tile([128, T], f32, tag="ps", name="ps")
                    for dc in range(8):
                        nc.tensor.matmul(
                            out=ps[:, 0:w],
                            lhsT=XB[dc][:, tcx * 128 : (tcx + 1) * 128],
                            rhs=vpans[dc][:, 0:w],
                            start=(dc == 0), stop=False,
                        )
                    nc.tensor.matmul(
                        out=ps[:, 0:w],
                        lhsT=onesrb_t[:, tcx * 128 : (tcx + 1) * 128],
                        rhs=vb[:, s0:s1],
                        start=False, stop=True,
                    )
                    nc.scalar.activation(VT[4 + tcx][:, s0:s1], ps[:, 0:w], AF.Copy)
            with tc.high_priority():
                for tcx in range(4):
                    for half in (0, 576):
                        nc.sync.dma_start(
                            out=rsv_in[l][half + tcx * 128 : half + (tcx + 1) * 128, :],
                            in_=VT[4 + tcx][:],
                        )
                for half in (0, 576):
                    nc.sync.dma_start(
                        out=rsv_in[l][half + 512 : half + 576, :], in_=VT[4][0:64, :]
                    )
            nc.gpsimd.collective_compute(
                "ReduceScatter", mybir.AluOpType.add, replica_groups=RG,
                ins=[rsv_in[l][:]], outs=[rsv_out[l][:]],
            )
            for pc in range(4, 8):  # local denominator ones-columns
                nc.gpsimd.memset(
                    VT[pc][:].rearrange("p (h c) -> p h c", c=65)[:, :, 64:65], 1.0
                )

            # ---------- Q projection ----------
            for ec in range(8):
                pan = wpanel(I["wq_pan"], l, ec)
                ps = pp.tile([128, T], f32, tag="ps", name="ps")
                for dc in range(8):
                    nc.tensor.matmul(
                        out=ps[:], lhsT=pan[:, dc, :], rhs=XB[dc][:],
                        start=(dc == 0), stop=(dc == 7),
                    )
                nc.scalar.activation(
                    Q[ec][:], ps[:], AF.Identity,
                    bias=par[:, PC_BQ + ec : PC_BQ + ec + 1], scale=1.0,
                )

            # ---------- attention: software-pipelined QK/exp ahead of AV ----------
            # units are chunk groups sharing one PSUM tile and ONE exp (the pb
            # bias is identical within each unit for both core parities). Each
            # chunk carries a query offset `off`: queries [0:off) cannot see
            # any key of that chunk (mask), so QK/exp/AV are trimmed to
            # columns [off:512). The first chunk of each phase covers the full
            # query range, so the PSUM accumulation group reset is complete.
            LOOKU = 5  # units of QK/exp lookahead ahead of the AV consumer

            def attn_pipeline(units, close_head):
                from collections import deque

                work = [(h, ui) for h in range(16) for ui in range(len(units))]
                nu = len(units)
                nch = sum(len(u) for u in units)
                pend = deque()
                avs = {}

                def drain_one():
                    h2, ui2, pts = pend.popleft()
                    if ui2 == 0:
                        avs[h2] = pav.tile([128, T], f32, tag="av", name="av")
                    base = sum(len(units[j]) for j in range(ui2))
                    for k, (c2, off2, ptk) in enumerate(pts):
                        nc.tensor.matmul(
                            out=avs[h2][0:65, off2:512],
                            lhsT=VT[c2][:, h2 * 65 : h2 * 65 + 65],
                            rhs=ptk,
                            start=(base + k == 0), stop=(base + k == nch - 1),
                        )
                    if ui2 == nu - 1:
                        close_head(h2, avs.pop(h2))

                for h, ui in work:
                    unit = units[ui]
                    par_, kc = h % 2, h // 2
                    rows = slice(par_ * 64, par_ * 64 + 64)
                    wtot = sum(512 - off for _, off in unit)
                    if wtot > 512:
                        sc = pp2.tile([128, 2 * T], f32, tag="ps2", name="ps2")
                        pt = prA.tile([128, 2 * T], bf16, tag="probs2", name="probs2")
                    else:
                        sc = pp.tile([128, T], f32, tag="ps", name="ps")
                        pt = prB.tile([128, T], bf16, tag="probs", name="probs")
                    col = 0
                    for c, off in unit:
                        w = 512 - off
                        nc.tensor.matmul(
                            out=sc[:, col : col + w],
                            lhsT=KH[kc][rows, c * 128 : (c + 1) * 128],
                            rhs=Q[kc][rows, off:512],
                            start=True, stop=True,
                        )
                        col += w
                    nc.scalar.activation(
                        pt[:, 0:wtot], sc[:, 0:wtot], AF.Exp,
                        bias=pb_t[unit[0][0]][:], scale=1.0,
                    )
                    pts = []
                    col = 0
                    for c, off in unit:
                        w = 512 - off
                        ptk = pt[:, col : col + w]
                        ext = SELEXT.get(c, 0) - off
                        if ext > 0:
                            nc.gpsimd.affine_select(
                                out=ptk[:, 0:ext], in_=ptk[:, 0:ext],
                                pattern=[[1, ext]], compare_op=ALU.is_ge,
                                fill=0.0, base=576 - c * 128 + off,
                                channel_multiplier=-1,
                            )
                        pts.append((c, off, ptk))
                        col += w
                    pend.append((h, ui, pts))
                    if len(pend) > LOOKU:
                        drain_one()
                while pend:
                    drain_one()

            attn_pipeline(
                [((4, 0), (5, 64)), ((6, 192), (7, 320))],
                lambda h, av: nc.vector.tensor_copy(AVL[h][0:65, :], av[0:65, :]),
            )

            # ---------- K/V fixup (consumes RS results) ----------
            for ec in range(8):
                ka = fx.tile([128, 576], bf16, tag="fxk", name="fxk")
                nc.sync.dma_start(out=ka[:], in_=rsk_out[l][ec * 128 : (ec + 1) * 128, :])
                nc.vector.tensor_sub(
                    KH[ec][:, 0:512], ka[:, 0:512], KH[ec][:, 512:1024]
                )
                nc.vector.tensor_sub(
                    KH[ec][:, 1024:1088], ka[:, 512:576], KH[ec][:, 512:576]
                )
            for pc in range(4):
                va = fx.tile([128, V_E], bf16, tag="fxv", name="fxv")
                nc.sync.dma_start(out=va[:], in_=rsv_out[l][pc * 128 : (pc + 1) * 128, :])
                nc.vector.tensor_sub(VT[pc][:], va[:], VT[4 + pc][:])
            va = fx.tile([128, V_E], bf16, tag="fxv", name="fxv")
            nc.sync.dma_start(out=va[0:64, :], in_=rsv_out[l][512:576, :])
            nc.vector.tensor_sub(VT[8][0:64, :], va[0:64, :], VT[4][0:64, :])
            for pc in (0, 1, 2, 3, 8):
                nc.gpsimd.memset(
                    VT[pc][:].rearrange("p (h c) -> p h c", c=65)[:, :, 64:65], 1.0
                )

            # ---------- attention phase B: remote chunks + deferred combine ----------
            def close_b(h, av2):
                avt = tps.tile([128, T], f32, tag="t512", name="t512")
                nc.vector.tensor_add(avt[0:65, :], av2[0:65, :], AVL[h][0:65, :])
                rr = rhp.tile([1, T], f32r, tag="rh", name="rht")
                with nc.allow_low_precision(reason="f32r is fp32-width storage"):
                    nc.vector.reciprocal(rr[:], avt[64:65, :])
                bc = pp.tile([128, T], f32, tag="ps", name="ps")
                nc.tensor.matmul(
                    out=bc[0:64, :], lhsT=r(ones1[:, 0:64]), rhs=r(rr[:]),
                    start=True, stop=True,
                )
                if h % 2 == 0:
                    nc.vector.tensor_mul(
                        OP[h // 2][0:64, :], avt[0:64, :], bc[0:64, :]
                    )
                else:
                    # stash in the (now free) AVL tile; DMA'd to OP rows 64:128
                    # after phase B so SP prefetching is not blocked mid-phase
                    nc.vector.tensor_mul(AVL[h][0:64, :], avt[0:64, :], bc[0:64, :])

            attn_pipeline([((0, 0), (1, 0)), ((2, 0), (3, 0)), ((8, 448),)], close_b)
            for h in range(1, 16, 2):
                nc.sync.dma_start(out=OP[h // 2][64:128, :], in_=AVL[h][0:64, :])

            def layernorm(src, gcol, bcol, dst, make_xb=False):
                mu = pp.tile([1, T], f32, tag="ps", name="ps")
                ms = pp.tile([1, T], f32, tag="ps", name="ps")
                for dc in range(8):
                    sq = tpr.tile([128, T], f32r, tag="sqr", name="sqr")
                    nc.scalar.square(sq[:], q32(src[dc][:]))
                    nc.tensor.matmul(
                        out=mu[:], lhsT=r(onesd_t[:]), rhs=r(src[dc][:]),
                        start=(dc == 0), stop=(dc == 7),
                    )
                    nc.tensor.matmul(
                        out=ms[:], lhsT=r(onesd_t[:]), rhs=r(sq[:]),
                        start=(dc == 0), stop=(dc == 7),
                    )
                mu_sb = sm.tile([1, T], f32r, tag="sm1", name="mu")
                nc.vector.tensor_copy(mu_sb[:], mu[:])
                t2 = sm.tile([1, T], f32, tag="sm1", name="t2")
                nc.vector.tensor_mul(t2[:], q32(mu_sb[:]), q32(mu_sb[:]))
                var = sm.tile([1, T], f32, tag="sm1", name="var")
                nc.vector.tensor_sub(var[:], ms[:], t2[:])
                nc.vector.tensor_scalar_add(var[:], var[:], EPS)
                std = sm.tile([1, T], f32, tag="sm1", name="std")
                nc.scalar.sqrt(std[:], var[:])
                rstd = sm.tile([1, T], f32r, tag="sm1", name="rstd")
                with nc.allow_low_precision(reason="f32r is fp32-width storage"):
                    nc.vector.reciprocal(rstd[:], std[:])
                mub = pp.tile([128, T], f32, tag="ps", name="ps")
                nc.tensor.matmul(
                    out=mub[:], lhsT=r(ones1[:]), rhs=r(mu_sb[:]), start=True, stop=True
                )
                rsb = pp.tile([128, T], f32, tag="ps", name="ps")
                nc.tensor.matmul(
                    out=rsb[:], lhsT=r(ones1[:]), rhs=r(rstd[:]), start=True, stop=True
                )
                rsb_sb = tps.tile([128, T], f32, tag="t512", name="t512")
                nc.vector.tensor_copy(rsb_sb[:], rsb[:])
                for dc in range(8):
                    t = tps.tile([128, T], f32, tag="t512", name="t512")
                    nc.vector.tensor_sub(t[:], q32(src[dc][:]), mub[:])
                    t2b = tps.tile([128, T], f32, tag="t512", name="t512")
                    nc.vector.tensor_mul(t2b[:], t[:], rsb_sb[:])
                    nc.scalar.activation(
                        dst[dc][:], t2b[:], AF.Identity,
                        bias=par[:, bcol + dc : bcol + dc + 1],
                        scale=par[:, gcol + dc : gcol + dc + 1],
                    )
                    if make_xb:
                        nc.scalar.activation(XB[dc][:], q32(dst[dc][:]), AF.Copy)

            # ---------- Wo + residual + LN1 ----------
            for ec in range(8):
                pan = wp.tile([128, 8, 128], bf16, tag="wpan2", name="wpan2")
                nc.sync.dma_start(out=pan[:], in_=I["wo_pan"][l, ec])
                ps = pp.tile([128, T], f32, tag="ps", name="ps")
                for dc in range(8):
                    nc.tensor.matmul(
                        out=ps[:], lhsT=pan[:, dc, :], rhs=OP[dc][:],
                        start=(dc == 0), stop=(dc == 7),
                    )
                nc.vector.scalar_tensor_tensor(
                    out=X2[ec][:], in0=ps[:], scalar=par[:, PC_BO + ec : PC_BO + ec + 1],
                    in1=q32(X[ec][:]), op0=ALU.add, op1=ALU.add,
                )
            layernorm(X2, PC_L1G, PC_L1B, X, make_xb=True)

            # ---------- FFN: W1 -> H (bf16), W2 accumulated in PSUM ----------
            for fc in range(32):
                pan = wp.tile([128, 8, 128], bf16, tag="wpan", name="wpan")
                nc.sync.dma_start(out=pan[:], in_=I["w1_pan"][l, fc])
                ps = pp.tile([128, T], f32, tag="ps", name="ps")
                for dc in range(8):
                    nc.tensor.matmul(
                        out=ps[:], lhsT=pan[:, dc, :], rhs=XB[dc][:],
                        start=(dc == 0), stop=(dc == 7),
                    )
                nc.scalar.activation(
                    HT[fc][:], ps[:], AF.Relu,
                    bias=par[:, PC_B1 + fc : PC_B1 + fc + 1], scale=1.0,
                )
            for ec in range(8):
                ps = pp.tile([128, T], f32, tag="ps", name="ps")
                for g in range(4):
                    pan = wp.tile([128, 8, 128], bf16, tag="wpan2", name="wpan2")
                    nc.sync.dma_start(out=pan[:], in_=I["w2_pan"][l, ec, g])
                    for k in range(8):
                        fc = g * 8 + k
                        nc.tensor.matmul(
                            out=ps[:], lhsT=pan[:, k, :], rhs=HT[fc][:],
                            start=(fc == 0), stop=(fc == 31),
                        )
                nc.vector.scalar_tensor_tensor(
                    out=X2[ec][:], in0=ps[:], scalar=par[:, PC_B2 + ec : PC_B2 + ec + 1],
                    in1=q32(X[ec][:]), op0=ALU.add, op1=ALU.add,
                )
            layernorm(X2, PC_L2G, PC_L2B, X, make_xb=(l < L - 1))

        for ec in range(8):
            nc.sync.dma_start(out=y[ec * 128 : (ec + 1) * 128, :], in_=q32(X[ec][:]))

    nc.compile()
    return nc


def _to_bf16(a):
    try:
        import ml_dtypes

        return a.astype(ml_dtypes.bfloat16)
    except ImportError:
        u = a.astype(np.float32).view(np.uint32)
        return ((u + 0x7FFF + ((u >> 16) & 1)) >> 16).astype(np.uint16)


def _host_prep(inputs):
    g = {}
    Wqkv = np.asarray(inputs["Wqkv"], np.float32)
    bqkv = np.asarray(inputs["bqkv"], np.float32)
    sc = 1.0 / np.sqrt(HD)
    wvT = np.zeros((L, D, V_E), np.float32)
    vbias = np.zeros((L, 1, V_E), np.float32)
    wq = np.zeros((L, D, D), np.float32)
    wk = np.zeros((L, D, D), np.float32)
    for l in range(L):
        Wq, Wk, Wv = Wqkv[l, 0:D], Wqkv[l, D : 2 * D], Wqkv[l, 2 * D :]
        bv = bqkv[l, 2 * D :]
        wq[l] = Wq.T * sc
        wk[l] = Wk.T
        for h in range(H):
            off = h * 65
            wvT[l, :, off : off + 64] = Wv.T[:, h * 64 : h * 64 + 64]
            vbias[l, 0, off : off + 64] = bv[h * 64 : h * 64 + 64]

    def pan5(wT):  # [L, D, M] -> [L, M/128, 128, 8, 128]
        Lx, Dx, M = wT.shape
        return np.ascontiguousarray(
            wT.reshape(Lx, 8, 128, M // 128, 128).transpose(0, 3, 2, 1, 4)
        )

    woT = np.asarray(inputs["Wo"], np.float32).transpose(0, 2, 1)
    w1T = np.asarray(inputs["W1"], np.float32).transpose(0, 2, 1)
    w2T = np.asarray(inputs["W2"], np.float32).transpose(0, 2, 1)
    g["wq_pan"] = _to_bf16(pan5(wq))
    g["wk_pan"] = _to_bf16(pan5(wk))
    g["wo_pan"] = _to_bf16(pan5(woT))
    g["w1_pan"] = _to_bf16(pan5(w1T))
    # w2: [L, DFF, D] -> [L, ec, g, 128, k, 128]
    w2p = np.empty((L, 8, 4, 128, 8, 128), np.float32)
    for ec in range(8):
        for gg in range(4):
            for k in range(8):
                fc = gg * 8 + k
                w2p[:, ec, gg, :, k, :] = w2T[
                    :, fc * 128 : (fc + 1) * 128, ec * 128 : (ec + 1) * 128
                ]
    g["w2_pan"] = _to_bf16(np.ascontiguousarray(w2p))
    g["wv_pan"] = _to_bf16(
        np.ascontiguousarray(wvT.reshape(L, 8, 128, V_E))
    )
    g["vbias"] = _to_bf16(vbias)

    par = np.zeros((L, 128, 96), np.float32)
    par[:, :, PC_BQ : PC_BQ + 8] = (bqkv[:, 0:D] * sc).reshape(L, 8, 128).transpose(0, 2, 1)
    par[:, :, PC_BK : PC_BK + 8] = bqkv[:, D : 2 * D].reshape(L, 8, 128).transpose(0, 2, 1)
    par[:, :, PC_BO : PC_BO + 8] = np.asarray(inputs["bo"], np.float32).reshape(L, 8, 128).transpose(0, 2, 1)
    par[:, :, PC_B2 : PC_B2 + 8] = np.asarray(inputs["b2"], np.float32).reshape(L, 8, 128).transpose(0, 2, 1)
    par[:, :, PC_L1G : PC_L1G + 8] = np.asarray(inputs["g1"], np.float32).reshape(L, 8, 128).transpose(0, 2, 1)
    par[:, :, PC_L1B : PC_L1B + 8] = np.asarray(inputs["be1"], np.float32).reshape(L, 8, 128).transpose(0, 2, 1)
    par[:, :, PC_L2G : PC_L2G + 8] = np.asarray(inputs["g2"], np.float32).reshape(L, 8, 128).transpose(0, 2, 1)
    par[:, :, PC_L2B : PC_L2B + 8] = np.asarray(inputs["be2"], np.float32).reshape(L, 8, 128).transpose(0, 2, 1)
    par[:, :, PC_B1 : PC_B1 + 32] = np.asarray(inputs["b1"], np.float32).reshape(L, 32, 128).transpose(0, 2, 1)
    g["par"] = par

    g["ones1"] = np.ones((1, 128), np.float32)
    g["onesd"] = np.full((128, 1), 1.0 / D, np.float32)
    g["onesrb"] = _to_bf16(np.ones((1, 512), np.float32))

    xb = np.asarray(inputs["x"], np.float32).transpose(1, 0, 2)
    in_maps = []
    for c in range(8):
        b, hh = c // 2, c % 2
        pb = np.zeros((NC, 128, 1), np.float32)
        if hh == 0:
            pb[0:4] = NEG
            pb[8, 64:128] = NEG
        else:
            pb[8] = NEG
        m = dict(g)
        m["x0"] = np.ascontiguousarray(xb[b, hh * T : (hh + 1) * T, :].T)
        m["pb"] = pb
        in_maps.append(m)
    return in_maps


def kernel(**inputs):
    from concourse.bass_utils import run_bass_kernel_spmd

    if "nc" not in _CACHE:
        _CACHE["nc"] = _build_program()
    nc = _CACHE["nc"]
    in_maps = _host_prep(inputs)
    res = run_bass_kernel_spmd(nc, in_maps, core_ids=list(range(8)))
    out = np.zeros((S, B, D), np.float32)
    for c in range(8):
        b, hh = c // 2, c % 2
        out[hh * T : (hh + 1) * T, b, :] = res.results[c]["y"].T
    return out
